# revision 1
# baseline (speedup 1.0000x reference)
"""Trainium2 Bass kernel for nn_KeyRecorder.

Math (reference):
  comp = LN(relu(obs @ W1 + b1)) * g1 + bl1          [B, T, R]
  past = max(comp[:, :-20:10, :], axis=time)          408 strided rows
  gmax = max(cummax(comp[:, -20:, :]), past)          [B, 20, R]
  out  = LN(relu(gmax @ W2 + b2)) * g2 + bl2          [B, 20, D]

Only 428 of the 4096 timesteps per batch element are ever consumed
(408 strided + last 20), so the host gathers exactly those rows,
transposes them to d-major layout, and ships ~1.75 MB/core instead of
16.8 MB/core.  Batch is sharded 2-per-core across 8 cores (pure data
parallel, no collectives).

LN1's affine (g1, bl1) is folded into W2/b2 on the host:
  max/cummax commute with x -> x*g1+bl1 elementwise when g1 >= 0
  (asserted), and (gmax*g1+bl1) @ W2 = gmax @ (g1[:,None]*W2) + bl1@W2.
"""

import os
import numpy as np

import concourse.bass as bass
import concourse.bacc as bacc
import concourse.mybir as mybir
import concourse.tile as tile
from concourse.bass_utils import run_bass_kernel_spmd

F32 = mybir.dt.float32
ALU = mybir.AluOpType
ACT = mybir.ActivationFunctionType
AX = mybir.AxisListType

B, T, D, R = 16, 4096, 512, 64
LOCAL, SR, EPS = 20, 10, 1e-5
N_CORES = 8
BPC = B // N_CORES            # batch elements per core
NSTR = (T - LOCAL + SR - 1) // SR   # 408 strided past rows
NSEL = NSTR + LOCAL           # 428 rows consumed per batch element
GRP = 448                     # per-batch group width in SBUF (428 padded)
NTOK = GRP * BPC              # 896 token columns per core
NTT = NTOK // 128             # 7 token tiles
DC = D // 128                 # 4 contraction chunks
NO = BPC * LOCAL              # 40 output rows per core

IDX = np.array(list(range(0, T - LOCAL, SR)) + list(range(T - LOCAL, T)))

_cache: dict = {}


def _build_program():
    """Build + compile the per-core Bass program once."""
    if "nc" in _cache:
        return _cache["nc"]

    nc = bacc.Bacc("TRN2", target_bir_lowering=False, debug=False,
                   enable_asserts=False)

    obsT_d = nc.dram_tensor("obsT", [DC, 128, NTOK], F32, kind="ExternalInput")
    w1_d = nc.dram_tensor("w1c", [DC, 128, R], F32, kind="ExternalInput")
    b1_d = nc.dram_tensor("b1col", [R, 1], F32, kind="ExternalInput")
    w2_d = nc.dram_tensor("w2f", [R, D], F32, kind="ExternalInput")
    b2_d = nc.dram_tensor("b2row", [1, D], F32, kind="ExternalInput")
    g2_d = nc.dram_tensor("g2b", [NO, D], F32, kind="ExternalInput")
    bl2_d = nc.dram_tensor("bl2b", [NO, D], F32, kind="ExternalInput")
    id_d = nc.dram_tensor("ident", [128, 128], F32, kind="ExternalInput")
    out_d = nc.dram_tensor("out", [NO, D], F32, kind="ExternalOutput")

    inv_r = 1.0 / R
    inv_d = 1.0 / D

    with tile.TileContext(nc) as tc:
        with (
            tc.tile_pool(name="const", bufs=1) as cpool,
            tc.tile_pool(name="work", bufs=4) as wpool,
            tc.tile_pool(name="stats", bufs=6) as spool,
            tc.tile_pool(name="ps_grp", bufs=2, space=bass.MemorySpace.PSUM) as pgrp,
            tc.tile_pool(name="ps_mm", bufs=3, space=bass.MemorySpace.PSUM) as pmm,
            tc.tile_pool(name="ps_tr", bufs=2, space=bass.MemorySpace.PSUM) as ptr,
            tc.tile_pool(name="ps_o", bufs=1, space=bass.MemorySpace.PSUM) as pout,
        ):
            # ---- load constants first (first matmul needs them) ----
            w1 = cpool.tile([128, DC, R], F32)
            for c in range(DC):
                nc.sync.dma_start(w1[:, c, :], w1_d[c])
            b1c = cpool.tile([R, 1], F32)
            nc.sync.dma_start(b1c[:], b1_d[:])
            ident = cpool.tile([128, 128], F32)
            nc.sync.dma_start(ident[:], id_d[:])
            ones1 = cpool.tile([1, 128], F32)
            nc.vector.memset(ones1[:], 1.0)
            w2 = cpool.tile([R, D], F32)
            nc.sync.dma_start(w2[:], w2_d[:])
            b2r = cpool.tile([1, D], F32)
            nc.sync.dma_start(b2r[:], b2_d[:])
            g2 = cpool.tile([NO, D], F32)
            nc.sync.dma_start(g2[:], g2_d[:])
            bl2 = cpool.tile([NO, D], F32)
            nc.sync.dma_start(bl2[:], bl2_d[:])

            # ---- input: one contiguous DMA per contraction chunk ----
            obsT = cpool.tile([128, DC, NTOK], F32)
            for c in range(DC):
                nc.sync.dma_start(obsT[:, c, :], obsT_d[c])

            compT = cpool.tile([R, NTOK], F32)   # LN'd comp, [r, t] layout

            # ---- stage 1: comp = LN(relu(obs @ W1 + b1)) ----
            # W1-stationary matmuls: compT_pre [r, t] in two PSUM groups
            # (512 + 384 cols), bias fused into the PSUM->SBUF copy.
            cpre = cpool.tile([R, NTOK], F32)
            for g, (lo, w) in enumerate(((0, 512), (512, 384))):
                pg = pgrp.tile([R, 512], F32, tag="pg")
                for c in range(DC):
                    nc.tensor.matmul(pg[:, 0:w], w1[:, c, :],
                                     obsT[:, c, lo:lo + w],
                                     start=(c == 0), stop=(c == DC - 1))
                nc.vector.tensor_scalar_add(cpre[:, lo:lo + w], pg[:, 0:w],
                                            b1c[:])
            for tt in range(NTT):
                # transpose 128-token slab to [t, r] for the row LN
                ps = pmm.tile([128, R], F32, tag="ps")
                nc.tensor.transpose(ps[:], cpre[:, bass.ts(tt, 128)],
                                    ident[0:R, 0:R])
                # relu + row-sum in one op
                xr = wpool.tile([128, R], F32, tag="xr")
                rsum = spool.tile([128, 1], F32, tag="rsum")
                nc.vector.tensor_scalar(xr[:], ps[:], 0.0, 0.0, ALU.max,
                                        ALU.add, accum_out=rsum[:])
                negmu = spool.tile([128, 1], F32, tag="negmu")
                nc.gpsimd.tensor_scalar_mul(negmu[:], rsum[:], -inv_r)
                xc = wpool.tile([128, R], F32, tag="xc")
                nc.vector.tensor_scalar_add(xc[:], xr[:], negmu[:])
                # squared sum: square on ACT with fused row-sum
                sq = wpool.tile([128, R], F32, tag="sq")
                ssq = spool.tile([128, 1], F32, tag="ssq")
                nc.scalar.activation(sq[:], xc[:], ACT.Square,
                                     accum_out=ssq[:])
                ssqe = spool.tile([128, 1], F32, tag="ssqe")
                nc.vector.tensor_scalar_add(ssqe[:], ssq[:], R * EPS)
                std = spool.tile([128, 1], F32, tag="std")
                nc.scalar.activation(std[:], ssqe[:], ACT.Sqrt,
                                     bias=0.0, scale=inv_r)
                rstd = spool.tile([128, 1], F32, tag="rstd")
                nc.vector.reciprocal(rstd[:], std[:])
                y = wpool.tile([128, R], F32, tag="y")
                nc.vector.tensor_scalar_mul(y[:], xc[:], rstd[:])

                # transpose to [r, t] for the time reductions
                pt = ptr.tile([R, 128], F32, tag="pt")
                nc.tensor.transpose(pt[:], y[:], ident[:])
                nc.vector.tensor_copy(compT[:, bass.ts(tt, 128)], pt[:])

            # ---- stage 2: strided max + seeded cummax (free-axis ops) ----
            past0 = spool.tile([R, 1], F32, tag="past0")
            nc.vector.reduce_max(past0[:], compT[:, 0:NSTR], axis=AX.X)
            past1 = spool.tile([R, 1], F32, tag="past1")
            nc.vector.reduce_max(past1[:], compT[:, GRP:GRP + NSTR], axis=AX.X)

            pa = cpool.tile([R, BPC, LOCAL], F32)
            pb = cpool.tile([R, BPC, LOCAL], F32)
            nc.vector.tensor_copy(pa[:, 0, :], compT[:, NSTR:NSEL])
            nc.vector.tensor_copy(pa[:, 1, :], compT[:, GRP + NSTR:GRP + NSEL])
            cur, nxt = pa, pb
            s = 1
            while s < LOCAL:
                nc.vector.tensor_tensor(nxt[:, :, s:], cur[:, :, s:],
                                        cur[:, :, :LOCAL - s], op=ALU.max)
                nc.vector.tensor_copy(nxt[:, :, 0:s], cur[:, :, 0:s])
                cur, nxt = nxt, cur
                s *= 2

            gmaxT = cpool.tile([R, NO], F32)
            nc.vector.tensor_scalar(gmaxT[:, 0:LOCAL], cur[:, 0, :],
                                    past0[:], None, ALU.max)
            nc.vector.tensor_scalar(gmaxT[:, LOCAL:NO], cur[:, 1, :],
                                    past1[:], None, ALU.max)

            # ---- stage 3: out = LN(relu(gmax @ W2' + b2')) * g2 + bl2 ----
            ps2 = pout.tile([NO, D], F32)
            nc.tensor.matmul(ps2[:], gmaxT[:], w2[:], start=True, stop=False)
            nc.tensor.matmul(ps2[:], ones1[:, 0:NO], b2r[:],
                             start=False, stop=True)

            xr2 = cpool.tile([NO, D], F32)
            rsum2 = spool.tile([NO, 1], F32, tag="rsum2")
            nc.vector.tensor_scalar(xr2[:], ps2[:], 0.0, 0.0, ALU.max,
                                    ALU.add, accum_out=rsum2[:])
            negmu2 = spool.tile([NO, 1], F32, tag="negmu2")
            nc.gpsimd.tensor_scalar_mul(negmu2[:], rsum2[:], -inv_d)
            xc2 = cpool.tile([NO, D], F32)
            nc.vector.tensor_scalar_add(xc2[:], xr2[:], negmu2[:])
            sq2 = cpool.tile([NO, D], F32)
            ssq2 = spool.tile([NO, 1], F32, tag="ssq2")
            nc.scalar.activation(sq2[:], xc2[:], ACT.Square,
                                 accum_out=ssq2[:])
            ssqe2 = spool.tile([NO, 1], F32, tag="ssqe2")
            nc.vector.tensor_scalar_add(ssqe2[:], ssq2[:], D * EPS)
            std2 = spool.tile([NO, 1], F32, tag="std2")
            nc.scalar.activation(std2[:], ssqe2[:], ACT.Sqrt,
                                 bias=0.0, scale=inv_d)
            rstd2 = spool.tile([NO, 1], F32, tag="rstd2")
            nc.vector.reciprocal(rstd2[:], std2[:])
            yn = cpool.tile([NO, D], F32)
            nc.vector.tensor_scalar_mul(yn[:], xc2[:], rstd2[:])
            yg = cpool.tile([NO, D], F32)
            nc.vector.tensor_mul(yg[:], yn[:], g2[:])
            out_sb = cpool.tile([NO, D], F32)
            nc.vector.tensor_add(out_sb[:], yg[:], bl2[:])

            nc.sync.dma_start(out_d[:], out_sb[:])

    nc.compile()
    _cache["nc"] = nc
    return nc


def _host_inputs(obs, W1, b1, ln1_g, ln1_b, W2, b2, ln2_g, ln2_b):
    obs = np.ascontiguousarray(np.asarray(obs, dtype=np.float32))
    W1 = np.asarray(W1, np.float32)
    b1 = np.asarray(b1, np.float32)
    ln1_g = np.asarray(ln1_g, np.float32)
    ln1_b = np.asarray(ln1_b, np.float32)
    W2 = np.asarray(W2, np.float32)
    b2 = np.asarray(b2, np.float32)
    ln2_g = np.asarray(ln2_g, np.float32)
    ln2_b = np.asarray(ln2_b, np.float32)

    # folding LN1's affine past the max/cummax requires monotonicity
    assert np.all(ln1_g >= 0), "ln1_g must be >= 0 for the affine fold"

    w1c = np.ascontiguousarray(W1.reshape(DC, 128, R))
    b1r = b1.reshape(R, 1)
    w2f = np.ascontiguousarray(ln1_g[:, None] * W2)
    b2f = (b2 + ln1_b @ W2).astype(np.float32).reshape(1, D)
    g2b = np.ascontiguousarray(np.broadcast_to(ln2_g, (NO, D)))
    bl2b = np.ascontiguousarray(np.broadcast_to(ln2_b, (NO, D)))
    ident = np.eye(128, dtype=np.float32)

    shared = {"w1c": w1c, "b1col": b1r, "w2f": w2f, "b2row": b2f,
              "g2b": g2b, "bl2b": bl2b, "ident": ident}
    in_maps = []
    for c in range(N_CORES):
        sel = obs[BPC * c:BPC * (c + 1)][:, IDX, :]        # [BPC, 428, 512]
        grp = np.zeros((BPC, GRP, D), np.float32)
        grp[:, :NSEL] = sel
        obsT = np.ascontiguousarray(grp.reshape(NTOK, D).T)  # [512, 896]
        in_maps.append({"obsT": obsT.reshape(DC, 128, NTOK), **shared})
    return in_maps


def _install_ntff_shim():
    """The agent image's antenv lacks axon_hooks; synthesize it so
    trace=True can reach the libaxon NTFF profiler (test-time only)."""
    import sys
    import types
    if "antenv.axon_hooks" in sys.modules:
        return True
    try:
        import antenv
        from trn_agent_boot.trn_boot import _ntff_profile_via_ctypes
    except ImportError:
        return False
    so_path = "/opt/axon/libaxon_pjrt.so"
    if not os.path.exists(so_path):
        return False
    hook = _ntff_profile_via_ctypes(so_path)
    mod = types.ModuleType("antenv.axon_hooks")
    mod._hook = hook
    mod.set_axon_ntff_profile_hook = lambda h: setattr(mod, "_hook", h)
    mod.get_axon_ntff_profile_hook = lambda: mod._hook
    sys.modules["antenv.axon_hooks"] = mod
    antenv.axon_hooks = mod
    return hook is not None


def kernel(obs_frames, W1, b1, ln1_g, ln1_b, W2, b2, ln2_g, ln2_b):
    nc = _build_program()
    in_maps = _host_inputs(obs_frames, W1, b1, ln1_g, ln1_b,
                           W2, b2, ln2_g, ln2_b)
    trace = bool(os.environ.get("BASS_TRACE"))
    if trace:
        trace = _install_ntff_shim()
        import concourse.bass_utils as _bu
        _bu.upload_artifacts = lambda tmpdir: f"local://{tmpdir}"
    res = run_bass_kernel_spmd(nc, in_maps, core_ids=list(range(N_CORES)),
                               trace=trace)
    _cache["last_result"] = res
    out = np.stack([res.results[c]["out"].reshape(BPC, LOCAL, D)
                    for c in range(N_CORES)])
    return out.reshape(B, LOCAL, D)



# revision 3
# speedup vs baseline: 1.3478x; 1.3478x over previous
"""Trainium2 Bass kernel for nn_KeyRecorder.

Math (reference):
  comp = LN(relu(obs @ W1 + b1)) * g1 + bl1          [B, T, R]
  past = max(comp[:, :-20:10, :], axis=time)          408 strided rows
  gmax = max(cummax(comp[:, -20:, :]), past)          [B, 20, R]
  out  = LN(relu(gmax @ W2 + b2)) * g2 + bl2          [B, 20, D]

Only 428 of the 4096 timesteps per batch element are consumed (408
strided + last 20); the host gathers those rows, transposes to d-major,
casts to fp16 and ships ~0.9 MB/core.  Batch sharded 2-per-core across
8 cores, no collectives.

LN1's affine (g1, bl1) is folded into W2/b2 on the host (valid since
g1 >= 0, asserted).  b1/b2 bias adds are folded into the matmuls as
rank-1 updates.  LN1 runs in token-partition layout with all 7 token
tiles' stats batched into [128, 7] ops; E[x^2]-mu^2 variance avoids the
centering pass.
"""

import os
import numpy as np

import concourse.bass as bass
import concourse.bacc as bacc
import concourse.mybir as mybir
import concourse.tile as tile
from concourse.bass_utils import run_bass_kernel_spmd

F32 = mybir.dt.float32
F16 = mybir.dt.float16
ALU = mybir.AluOpType
ACT = mybir.ActivationFunctionType
AX = mybir.AxisListType

B, T, D, R = 16, 4096, 512, 64
LOCAL, SR, EPS = 20, 10, 1e-5
N_CORES = 8
BPC = B // N_CORES            # batch elements per core
NSTR = (T - LOCAL + SR - 1) // SR   # 408 strided past rows
NSEL = NSTR + LOCAL           # 428 rows consumed per batch element
GRP = 448                     # per-batch group width in SBUF (428 padded)
NTOK = GRP * BPC              # 896 token columns per core
NTT = NTOK // 128             # 7 token tiles
DC = D // 128                 # 4 contraction chunks
NO = BPC * LOCAL              # 40 output rows per core
CA = 512                      # token cols in group A
CB = NTOK - CA                # 384 token cols in group B

IDX = np.array(list(range(0, T - LOCAL, SR)) + list(range(T - LOCAL, T)))

_cache: dict = {}


def _build_program():
    """Build + compile the per-core Bass program once."""
    if "nc" in _cache:
        return _cache["nc"]

    nc = bacc.Bacc("TRN2", target_bir_lowering=False, debug=False,
                   enable_asserts=False)

    # cpack: w1 [:, 0:256] ([128, 4, 64] chunk-major), w2 rows 0:64 at
    # [*, 256:768], identity at [:, 768:896]
    cpack_d = nc.dram_tensor("cpack", [128, 896], F16, kind="ExternalInput")
    # rowc: b1 cols 0:64, b2 cols 64:576
    rowc_d = nc.dram_tensor("rowc", [1, 576], F16, kind="ExternalInput")
    obsA_d = nc.dram_tensor("obsA", [128, DC, CA], F16, kind="ExternalInput")
    obsB_d = nc.dram_tensor("obsB", [128, DC, CB], F16, kind="ExternalInput")
    g2bl2_d = nc.dram_tensor("g2bl2", [NO, 2 * D], F32, kind="ExternalInput")
    out_d = nc.dram_tensor("out16", [NO, D], F16, kind="ExternalOutput")

    inv_r = 1.0 / R
    inv_d = 1.0 / D

    with tile.TileContext(nc) as tc:
        with (
            tc.tile_pool(name="const", bufs=1) as cpool,
            tc.tile_pool(name="ps", bufs=1, space=bass.MemorySpace.PSUM) as pp,
        ):
            # ---- DMAs: sync queue gets cpack+obsA, scalar queue the rest
            cpack = cpool.tile([128, 896], F16)
            nc.sync.dma_start(cpack[:], cpack_d[:])
            rowc = cpool.tile([1, 576], F16)
            nc.scalar.dma_start(rowc[:], rowc_d[:])
            obsA = cpool.tile([128, DC, CA], F16)
            nc.sync.dma_start(obsA[:], obsA_d[:])
            obsB = cpool.tile([128, DC, CB], F16)
            nc.scalar.dma_start(obsB[:], obsB_d[:])
            g2bl2 = cpool.tile([NO, 2 * D], F32)
            nc.scalar.dma_start(g2bl2[:], g2bl2_d[:])

            ones16 = cpool.tile([1, CA], F16)
            nc.vector.memset(ones16[:], 1.0)

            identv = cpack[:, 768:896]

            # ---- stage 1 matmuls: pre[r, t] = W1^T obs + b1 (rank-1) ----
            pgA = pp.tile([R, CA], F32)
            for c in range(DC):
                nc.tensor.matmul(pgA[:], cpack[:, c * 64:(c + 1) * 64],
                                 obsA[:, c, :], start=(c == 0), stop=False)
            nc.tensor.matmul(pgA[:], rowc[0:1, 0:R], ones16[0:1, 0:CA],
                             start=False, stop=True)
            pgB = pp.tile([R, CB], F32)
            for c in range(DC):
                nc.tensor.matmul(pgB[:], cpack[:, c * 64:(c + 1) * 64],
                                 obsB[:, c, :], start=(c == 0), stop=False)
            nc.tensor.matmul(pgB[:], rowc[0:1, 0:R], ones16[0:1, 0:CB],
                             start=False, stop=True)

            # ---- relu into fp16 [r, t] ----
            xr = cpool.tile([R, NTOK], F16)
            nc.vector.tensor_scalar(xr[:, 0:CA], pgA[:], 0.0, None, ALU.max)
            nc.vector.tensor_scalar(xr[:, CA:NTOK], pgB[:], 0.0, None,
                                    ALU.max)

            # ---- transpose to token-partition layout [t, tile, r] ----
            xrT = pp.tile([128, NTT, R], F16)
            for tt in range(NTT):
                nc.tensor.transpose(xrT[:, tt, :], xr[:, bass.ts(tt, 128)],
                                    identv[0:R, 0:R])

            # ---- batched LN1 stats across all 7 tiles: [128, 7] ----
            sq16 = cpool.tile([128, NTT, R], F16)
            nc.scalar.activation(sq16[:], xrT[:], ACT.Square)
            rsum = cpool.tile([128, NTT], F32)
            nc.vector.reduce_sum(rsum[:], xrT[:], axis=AX.X)
            ssq = cpool.tile([128, NTT], F32)
            nc.vector.reduce_sum(ssq[:], sq16[:], axis=AX.X)
            negmu = cpool.tile([128, NTT], F32)
            nc.gpsimd.tensor_scalar_mul(negmu[:], rsum[:], -inv_r)
            vs = cpool.tile([128, NTT], F32)
            nc.vector.tensor_scalar(vs[:], ssq[:], inv_r, EPS, ALU.mult,
                                    ALU.add)
            musq = cpool.tile([128, NTT], F32)
            nc.scalar.activation(musq[:], negmu[:], ACT.Square)
            var = cpool.tile([128, NTT], F32)
            nc.vector.tensor_tensor(var[:], vs[:], musq[:], op=ALU.subtract)
            std = cpool.tile([128, NTT], F32)
            nc.scalar.activation(std[:], var[:], ACT.Sqrt)
            rstd = cpool.tile([128, NTT], F32)
            nc.vector.reciprocal(rstd[:], std[:])

            # ---- per-tile affine (x - mu) * rstd, back to [r, t] ----
            y16 = cpool.tile([128, NTT, R], F16)
            for tt in range(NTT):
                nc.vector.tensor_scalar(y16[:, tt, :], xrT[:, tt, :],
                                        negmu[:, tt:tt + 1],
                                        rstd[:, tt:tt + 1], ALU.add, ALU.mult)
            compT = pp.tile([R, NTOK], F16)
            for tt in range(NTT):
                nc.tensor.transpose(compT[:, bass.ts(tt, 128)], y16[:, tt, :],
                                    identv[:])

            # ---- stage 2: strided max + seeded cummax ----
            past0 = cpool.tile([R, 1], F32)
            nc.vector.reduce_max(past0[:], compT[:, 0:NSTR], axis=AX.X)
            past1 = cpool.tile([R, 1], F32)
            nc.vector.reduce_max(past1[:], compT[:, GRP:GRP + NSTR], axis=AX.X)

            pa = cpool.tile([R, BPC, LOCAL], F16)
            pb = cpool.tile([R, BPC, LOCAL], F16)
            nc.vector.tensor_copy(pa[:, 0, :], compT[:, NSTR:NSEL])
            nc.vector.tensor_copy(pa[:, 1, :], compT[:, GRP + NSTR:GRP + NSEL])
            cur, nxt = pa, pb
            s = 1
            while s < LOCAL:
                nc.vector.tensor_tensor(nxt[:, :, s:], cur[:, :, s:],
                                        cur[:, :, :LOCAL - s], op=ALU.max)
                nc.vector.tensor_copy(nxt[:, :, 0:s], cur[:, :, 0:s])
                cur, nxt = nxt, cur
                s *= 2

            gmax16 = cpool.tile([R, NO], F16)
            nc.vector.tensor_scalar(gmax16[:, 0:LOCAL], cur[:, 0, :],
                                    past0[:], None, ALU.max)
            nc.vector.tensor_scalar(gmax16[:, LOCAL:NO], cur[:, 1, :],
                                    past1[:], None, ALU.max)

            # ---- stage 3: out = LN(relu(gmax @ W2' + b2')) * g2 + bl2 ----
            ps2 = pp.tile([NO, D], F32)
            nc.tensor.matmul(ps2[:], gmax16[:], cpack[0:R, 256:768],
                             start=True, stop=False)
            nc.tensor.matmul(ps2[:], ones16[0:1, 0:NO], rowc[0:1, 64:576],
                             start=False, stop=True)

            xr2 = cpool.tile([NO, D], F32)
            rsum2 = cpool.tile([NO, 1], F32)
            nc.vector.tensor_scalar(xr2[:], ps2[:], 0.0, 0.0, ALU.max,
                                    ALU.add, accum_out=rsum2[:])
            sq2 = cpool.tile([NO, D], F32)
            ssq2 = cpool.tile([NO, 1], F32)
            nc.scalar.activation(sq2[:], xr2[:], ACT.Square,
                                 accum_out=ssq2[:])
            negmu2 = cpool.tile([NO, 1], F32)
            nc.gpsimd.tensor_scalar_mul(negmu2[:], rsum2[:], -inv_d)
            vs2 = cpool.tile([NO, 1], F32)
            nc.vector.tensor_scalar(vs2[:], ssq2[:], inv_d, EPS, ALU.mult,
                                    ALU.add)
            musq2 = cpool.tile([NO, 1], F32)
            nc.scalar.activation(musq2[:], negmu2[:], ACT.Square)
            var2 = cpool.tile([NO, 1], F32)
            nc.vector.tensor_tensor(var2[:], vs2[:], musq2[:],
                                    op=ALU.subtract)
            std2 = cpool.tile([NO, 1], F32)
            nc.scalar.activation(std2[:], var2[:], ACT.Sqrt)
            rstd2 = cpool.tile([NO, 1], F32)
            nc.vector.reciprocal(rstd2[:], std2[:])
            y2 = cpool.tile([NO, D], F32)
            nc.vector.tensor_scalar(y2[:], xr2[:], negmu2[:], rstd2[:],
                                    ALU.add, ALU.mult)
            yg = cpool.tile([NO, D], F32)
            nc.vector.tensor_tensor(yg[:], y2[:], g2bl2[:, 0:D], op=ALU.mult)
            out_sb = cpool.tile([NO, D], F16)
            nc.vector.tensor_tensor(out_sb[:], yg[:], g2bl2[:, D:2 * D],
                                    op=ALU.add)

            nc.sync.dma_start(out_d[:], out_sb[:])

    nc.compile()
    _cache["nc"] = nc
    return nc


def _host_inputs(obs, W1, b1, ln1_g, ln1_b, W2, b2, ln2_g, ln2_b):
    obs = np.ascontiguousarray(np.asarray(obs, dtype=np.float32))
    W1 = np.asarray(W1, np.float32)
    b1 = np.asarray(b1, np.float32)
    ln1_g = np.asarray(ln1_g, np.float32)
    ln1_b = np.asarray(ln1_b, np.float32)
    W2 = np.asarray(W2, np.float32)
    b2 = np.asarray(b2, np.float32)
    ln2_g = np.asarray(ln2_g, np.float32)
    ln2_b = np.asarray(ln2_b, np.float32)

    # folding LN1's affine past the max/cummax requires monotonicity
    assert np.all(ln1_g >= 0), "ln1_g must be >= 0 for the affine fold"

    cpack = np.zeros((128, 896), np.float16)
    cpack[:, 0:256] = W1.reshape(DC, 128, R).transpose(1, 0, 2).reshape(
        128, DC * R)
    cpack[0:R, 256:768] = (ln1_g[:, None] * W2).astype(np.float16)
    cpack[:, 768:896] = np.eye(128, dtype=np.float16)

    rowc = np.zeros((1, 576), np.float16)
    rowc[0, 0:R] = b1.astype(np.float16)
    rowc[0, R:R + D] = (b2 + ln1_b @ W2).astype(np.float16)

    g2bl2 = np.zeros((NO, 2 * D), np.float32)
    g2bl2[:, 0:D] = ln2_g
    g2bl2[:, D:2 * D] = ln2_b

    shared = {"cpack": cpack, "rowc": rowc, "g2bl2": g2bl2}
    in_maps = []
    for c in range(N_CORES):
        sel = obs[BPC * c:BPC * (c + 1)][:, IDX, :]        # [BPC, 428, 512]
        grp = np.zeros((BPC, GRP, D), np.float16)
        grp[:, :NSEL] = sel
        obsT = grp.reshape(NTOK, D).T                      # [512, 896] fp16
        arr4 = obsT.reshape(DC, 128, NTOK).transpose(1, 0, 2)  # [128, 4, 896]
        obsA = np.ascontiguousarray(arr4[:, :, 0:CA])
        obsB = np.ascontiguousarray(arr4[:, :, CA:NTOK])
        in_maps.append({"obsA": obsA, "obsB": obsB, **shared})
    return in_maps


def _install_ntff_shim():
    """The agent image's antenv lacks axon_hooks; synthesize it so
    trace=True can reach the libaxon NTFF profiler (test-time only)."""
    import sys
    import types
    if "antenv.axon_hooks" in sys.modules:
        return True
    try:
        import antenv
        from trn_agent_boot.trn_boot import _ntff_profile_via_ctypes
    except ImportError:
        return False
    so_path = "/opt/axon/libaxon_pjrt.so"
    if not os.path.exists(so_path):
        return False
    hook = _ntff_profile_via_ctypes(so_path)
    mod = types.ModuleType("antenv.axon_hooks")
    mod._hook = hook
    mod.set_axon_ntff_profile_hook = lambda h: setattr(mod, "_hook", h)
    mod.get_axon_ntff_profile_hook = lambda: mod._hook
    sys.modules["antenv.axon_hooks"] = mod
    antenv.axon_hooks = mod
    return hook is not None


def kernel(obs_frames, W1, b1, ln1_g, ln1_b, W2, b2, ln2_g, ln2_b):
    nc = _build_program()
    in_maps = _host_inputs(obs_frames, W1, b1, ln1_g, ln1_b,
                           W2, b2, ln2_g, ln2_b)
    trace = bool(os.environ.get("BASS_TRACE"))
    if trace:
        trace = _install_ntff_shim()
        import concourse.bass_utils as _bu
        _bu.upload_artifacts = lambda tmpdir: f"local://{tmpdir}"
    res = run_bass_kernel_spmd(nc, in_maps, core_ids=list(range(N_CORES)),
                               trace=trace)
    _cache["last_result"] = res
    out = np.stack([res.results[c]["out16"].astype(np.float32)
                    .reshape(BPC, LOCAL, D) for c in range(N_CORES)])
    return out.reshape(B, LOCAL, D)
